# revision 4
# baseline (speedup 1.0000x reference)
"""Device-kernel builder + host prep for nn_GatedConvNeXt3DBlock on 8 trn2 cores.

Sharding: channels (16/core) for the FFT conv + scan; one AllToAll to
token blocks (b, h-band) for Wo/LN/MLP/residual.
"""
import numpy as np
import ml_dtypes

import concourse.bass as bass
import concourse.tile as tile
import concourse.mybir as mybir
from concourse.masks import make_identity

F32 = mybir.dt.float32
BF16 = mybir.dt.bfloat16
AF = mybir.ActivationFunctionType
OP = mybir.AluOpType
BF = ml_dtypes.bfloat16

B, T, H, W, C = 2, 16, 56, 56, 128
HID = 4 * C
S = 16            # channels per core
NCORES = 8
N = B * T * H * W          # 100352 tokens
FW = 29                    # rfft bins along W
NTOK = N // NCORES         # 12544 tokens per core in phase 3
HB = 14                    # h rows per (b, hg) block
NC3 = S * B * 4 * H        # 7168 free cols in T-stage layout

_OFF = {}

def _build_dft_cols():
    cols = []
    off = 0

    def put(name, mat, rows0=0, dup64=False):
        nonlocal off
        a = np.zeros((128, mat.shape[1]), np.float32)
        a[rows0:rows0 + mat.shape[0]] = mat
        if dup64:
            a[64:64 + mat.shape[0]] = mat
        _OFF[name] = (off, mat.shape[1])
        cols.append(a)
        off += mat.shape[1]

    w = np.arange(W); f = np.arange(FW)
    th = 2 * np.pi * np.outer(w, f) / W
    put("A1", np.concatenate([np.cos(th), -np.sin(th)], axis=1))  # [56, 58]

    h = np.arange(H)
    thh = 2 * np.pi * np.outer(h, h) / H
    CH, SH = np.cos(thh), np.sin(thh)
    for nm, m in [("CH", CH), ("SH", SH), ("SnH", -SH),
                  ("CHi", CH / H), ("SHi", SH / H), ("SnHi", -SH / H)]:
        put(nm, m, dup64=True)

    t = np.arange(T)
    tht = 2 * np.pi * np.outer(t, t) / T
    Ct, St = np.cos(tht), np.sin(tht)

    def bd(m):
        out = np.zeros((128, 128), np.float32)
        for l in range(8):
            out[l*16:(l+1)*16, l*16:(l+1)*16] = m
        return out

    for nm, m in [("Ctb", bd(Ct)), ("Stb", bd(St)), ("Sntb", bd(-St)),
                  ("Ctbi", bd(Ct / T)), ("Stbi", bd(St / T)), ("Sntbi", bd(-St / T))]:
        put(nm, m)

    a_f = np.full(FW, 2.0); a_f[0] = 1.0; a_f[28] = 1.0
    thw = 2 * np.pi * np.outer(f, w) / W
    m = np.zeros((64, W), np.float32)
    m[0:29] = (a_f[:, None] * np.cos(thw)) / W
    m[32:61] = (-a_f[:, None] * np.sin(thw)) / W
    put("CWst", m)
    return np.concatenate(cols, axis=1)

DFTM = _build_dft_cols()
DFT_COLS = DFTM.shape[1]


def _kernel_to_freq(kernel):
    c, kt, kh, kw = kernel.shape
    ti = (np.arange(kt) - kt // 2) % T
    hi = (np.arange(kh) - kh // 2) % H
    wi = (np.arange(kw) - kw // 2) % W
    padded = np.zeros((c, T, H, W), np.float32)
    tt, th_, tw = np.meshgrid(ti, hi, wi, indexing="ij")
    padded[:, tt, th_, tw] = kernel
    return np.fft.fftn(padded, axes=(1, 2, 3))


def host_prep(inputs):
    x = np.asarray(inputs["x"], np.float32)
    Wg = np.asarray(inputs["Wg"], np.float32); bg = np.asarray(inputs["bg"], np.float32)
    Wh = np.asarray(inputs["Wh"], np.float32); bh = np.asarray(inputs["bh"], np.float32)
    Wo = np.asarray(inputs["Wo"], np.float32); bo = np.asarray(inputs["bo"], np.float32)
    W1 = np.asarray(inputs["W1"], np.float32); b1 = np.asarray(inputs["b1"], np.float32)
    W2 = np.asarray(inputs["W2"], np.float32); b2 = np.asarray(inputs["b2"], np.float32)
    gamma = np.asarray(inputs["gamma"], np.float32)
    ln_s = np.asarray(inputs["ln_scale"], np.float32)
    ln_b = np.asarray(inputs["ln_bias"], np.float32)

    xt = np.ascontiguousarray(x.reshape(N, C).T).astype(BF)

    gkf = _kernel_to_freq(np.asarray(inputs["gate_kernel"], np.float32))[:, :, :, :FW]
    hkf = _kernel_to_freq(np.asarray(inputs["hidden_kernel"], np.float32))[:, :, :, :FW]

    W2g = W2 * gamma[None, :]
    b2g = b2 * gamma
    w2l = np.concatenate([W2g[j*128:(j+1)*128, :] for j in range(4)], axis=1)
    dftm = DFTM.astype(BF)

    in_maps = []
    for i in range(NCORES):
        cs = slice(S * i, S * (i + 1))
        wgh = np.concatenate([Wg[:, cs], Wh[:, cs]], axis=1).astype(BF)
        bgh = np.concatenate([bg[cs], bh[cs]])

        # kf[conv, ri, p=(l*16+ft), (c, b, g, fh)]
        kf = np.zeros((2, 2, 128, S, B, 4, H), np.float32)
        for ci, kfull in enumerate((gkf, hkf)):
            sub = kfull[cs]                       # [S, T, H, FW]
            for l in range(8):
                for g in range(4):
                    fw = g * 8 + l
                    if fw >= FW:
                        continue
                    v = sub[:, :, :, fw]          # [S, T, H]
                    for ri, vv in ((0, v.real), (1, v.imag)):
                        blk = vv.transpose(1, 0, 2).astype(np.float32)  # [T, S, H]
                        for ft in range(T):
                            kf[ci, ri, l*16+ft, :, 0, g, :] = blk[ft]
                            kf[ci, ri, l*16+ft, :, 1, g, :] = blk[ft]
        kf = np.ascontiguousarray(kf.reshape(2, 2, 128, NC3)).astype(BF)

        b_i, hg = i // 4, i % 4
        xslab = x[b_i, :, HB*hg:HB*(hg+1), :, :]
        xres = np.ascontiguousarray(
            xslab.transpose(2, 1, 0, 3).reshape(NTOK, C)).astype(np.float32)

        biases = np.zeros((128, 13), np.float32)
        biases[0:32, 0] = bgh
        biases[:, 1] = bo
        biases[:, 2:6] = b1.reshape(4, 128).T
        biases[:, 6] = b2g
        biases[:, 7] = ln_s
        biases[:, 8] = ln_b
        biases[:, 9:13] = 1.702 * biases[:, 2:6]

        in_maps.append({
            "xt": xt, "xres": xres, "wgh": wgh, "kf": kf,
            "wo": Wo.astype(BF), "w1": W1.astype(BF), "w2l": w2l.astype(BF),
            "dftm": dftm, "biases": biases,
        })
    return in_maps


def assemble_output(results):
    out = np.empty((B, T, H, W, C), np.float32)
    for i in range(NCORES):
        y = np.asarray(results[i]["y"])
        b_i, hg = i // 4, i % 4
        out[b_i, :, HB*hg:HB*(hg+1), :, :] = (
            y.reshape(W, HB, T, C).transpose(2, 1, 0, 3))
    return out


def build_nc():
    nc = bass.Bass()
    xt_d = nc.dram_tensor("xt", [C, N], BF16, kind="ExternalInput")
    xres_d = nc.dram_tensor("xres", [NTOK, C], F32, kind="ExternalInput")
    wgh_d = nc.dram_tensor("wgh", [C, 32], BF16, kind="ExternalInput")
    kf_d = nc.dram_tensor("kf", [2, 2, 128, NC3], BF16, kind="ExternalInput")
    wo_d = nc.dram_tensor("wo", [C, C], BF16, kind="ExternalInput")
    w1_d = nc.dram_tensor("w1", [C, HID], BF16, kind="ExternalInput")
    w2l_d = nc.dram_tensor("w2l", [C, HID], BF16, kind="ExternalInput")
    dftm_d = nc.dram_tensor("dftm", [128, DFT_COLS], BF16, kind="ExternalInput")
    bias_d = nc.dram_tensor("biases", [128, 13], F32, kind="ExternalInput")
    y_d = nc.dram_tensor("y", [NTOK, C], F32, kind="ExternalOutput")

    G_d = nc.dram_tensor("Gs", [32, N], BF16)
    Gsp_d = nc.dram_tensor("Gsp", [2, W, S * B * H * T], BF16)
    a2a_in_d = nc.dram_tensor("a2a_in", [NCORES, S, NTOK], BF16)
    a2a_out_d = nc.dram_tensor("a2a_out", [C, NTOK], BF16)

    from contextlib import ExitStack
    with tile.TileContext(nc) as tc, ExitStack() as _ctx:
        consts = _ctx.enter_context(tc.tile_pool(name="consts", bufs=1))
        dft = consts.tile([128, DFT_COLS], BF16)
        nc.sync.dma_start(dft[:], dftm_d[:])
        bia = consts.tile([128, 13], F32)
        nc.sync.dma_start(bia[:], bias_d[:])
        wgh = consts.tile([C, 32], BF16)
        nc.sync.dma_start(wgh[:], wgh_d[:])
        ident = consts.tile([128, 128], BF16)
        make_identity(nc, ident[:])

        def M(name):
            o, n_ = _OFF[name]
            return dft[:, o:o + n_]

        # ---------------- Phase 1 ----------------
        with (
            tc.tile_pool(name="p1x", bufs=3) as p1x,
            tc.tile_pool(name="p1g", bufs=3) as p1g,
            tc.tile_pool(name="p1ps", bufs=2, space="PSUM") as p1ps,
        ):
            CHT = 2048
            for k in range(N // CHT):
                xtile = p1x.tile([C, CHT], BF16, tag="xt")
                nc.sync.dma_start(xtile[:], xt_d[:, k*CHT:(k+1)*CHT])
                ps = p1ps.tile([32, CHT], F32, tag="ps")
                for s_ in range(4):
                    nc.tensor.matmul(ps[:, s_*512:(s_+1)*512], wgh[:],
                                     xtile[:, s_*512:(s_+1)*512],
                                     start=True, stop=True)
                gt = p1g.tile([32, CHT], BF16, tag="gt")
                for s_ in range(4):
                    nc.scalar.activation(gt[:, s_*512:(s_+1)*512],
                                         ps[:, s_*512:(s_+1)*512],
                                         AF.Identity, bias=bia[0:32, 0:1])
                nc.sync.dma_start(G_d[:, k*CHT:(k+1)*CHT], gt[:])

        # ---------------- Phase 2 ----------------
        # pool discipline: odd stages on left stack, even on right; LIFO per side
        psc = tc.alloc_tile_pool(name="scbuf", bufs=1, side="left")
        g_sc = psc.tile([128, NTOK], BF16, tag="gsc")
        h_sc = psc.tile([128, NTOK], BF16, tag="hsc")

        for conv in range(2):
            pin = tc.alloc_tile_pool(name=f"cin{conv}", bufs=3, side="left")

            # --- S1: W-fwd -> Z1 [58 | c,b,(t h)]
            pZ1 = tc.alloc_tile_pool(name=f"Z1_{conv}", bufs=1, side="left")
            Z1 = pZ1.tile([58, S, B, T * H], BF16)
            with tc.tile_pool(name="s1ps", bufs=2, space="PSUM") as pps:
                for c in range(S):
                    for b in range(B):
                        xw = pin.tile([W, T * H], BF16, tag="xw")
                        src = G_d[conv*16 + c, b*T*H*W:(b+1)*T*H*W].rearrange(
                            "(t h w) -> w (t h)", t=T, h=H, w=W)
                        nc.sync.dma_start(xw[:], src)
                        ps = pps.tile([58, 1024], F32, tag="ps")
                        nc.tensor.matmul(ps[:, 0:448], M("A1")[0:56], xw[:, 0:448],
                                         start=True, stop=True)
                        nc.tensor.matmul(ps[:, 512:960], M("A1")[0:56], xw[:, 448:896],
                                         start=True, stop=True)
                        nc.scalar.activation(
                            Z1[:, c, b, 0:448], ps[:, 0:448], AF.Copy)
                        nc.scalar.activation(
                            Z1[:, c, b, 448:896], ps[:, 512:960], AF.Copy)

            # --- R1 -> Z2 [120=(t-par,h) | c,b,tp8,58]
            pZ2 = tc.alloc_tile_pool(name=f"Z2_{conv}", bufs=1, side="right")
            Z2 = pZ2.tile([120, 2, S * B * 8, FW], BF16)
            with tc.tile_pool(name="r1ps", bufs=2, space="PSUM") as pps:
                for c in range(S):
                    for b in range(B):
                        ps = pps.tile([120, 8, 58], BF16, tag="ps")
                        for tp in range(8):
                            nc.tensor.transpose(
                                ps[0:56, tp, :], Z1[:, c, b, (2*tp)*H:(2*tp+1)*H],
                                ident[0:58, 0:58])
                            nc.tensor.transpose(
                                ps[64:120, tp, :], Z1[:, c, b, (2*tp+1)*H:(2*tp+2)*H],
                                ident[0:58, 0:58])
                        srcv = ps[:].rearrange("p t (r f) -> p t r f", r=2)
                        cb0 = (c * B + b) * 8
                        dstv = Z2[:].rearrange("p r x f -> p x r f")[:, cb0:cb0+8]
                        nc.vector.tensor_copy(dstv[0:56], srcv[0:56])
                        nc.vector.tensor_copy(dstv[64:120], srcv[64:120])

            # --- S2: H-fwd -> Z3 [56 | ri2, c,b,t16,fw29]
            pZ1.release()
            pZ3 = tc.alloc_tile_pool(name=f"Z3_{conv}", bufs=1, side="left")
            Z3 = pZ3.tile([56, 2, S * B, FW, T], BF16)
            with tc.tile_pool(name="s2ps", bufs=2, space="PSUM") as pps:
                for par in range(2):
                    pb = 0 if par == 0 else 64
                    for ch in range(16):
                        re = Z2[pb:pb+56, 0, ch*16:(ch+1)*16, :]
                        im = Z2[pb:pb+56, 1, ch*16:(ch+1)*16, :]
                        psr = pps.tile([56, 512], F32, tag="r")
                        psi = pps.tile([56, 512], F32, tag="i")
                        nc.tensor.matmul(psr[:, 0:464], M("CH")[pb:pb+56], re,
                                         start=True, stop=False)
                        nc.tensor.matmul(psr[:, 0:464], M("SH")[pb:pb+56], im,
                                         start=False, stop=True)
                        nc.tensor.matmul(psi[:, 0:464], M("SnH")[pb:pb+56], re,
                                         start=True, stop=False)
                        nc.tensor.matmul(psi[:, 0:464], M("CH")[pb:pb+56], im,
                                         start=False, stop=True)
                        for ri, psx in ((0, psr), (1, psi)):
                            dst = Z3[:, ri, ch*2:(ch+1)*2, :, par::2]
                            nc.scalar.activation(
                                dst.rearrange("p cb f t -> p cb t f"),
                                psx[:, 0:464].rearrange(
                                    "p (cb t f) -> p cb t f", cb=2, t=8), AF.Copy)

            # --- R2 -> Z4 [128=(l8,t16) | ri2, (c b), g4, fh56]
            pZ2.release()
            pZ4 = tc.alloc_tile_pool(name=f"Z4_{conv}", bufs=1, side="right")
            Z4 = pZ4.tile([128, 2, S * B, 4, H], BF16)
            nc.vector.memset(
                Z4[:].rearrange("p r cb g f -> p (r cb) g f")[64:128, :, 3, :], 0.0)

            with tc.tile_pool(name="r2ps", bufs=2, space="PSUM") as pps:
                for cb in range(S * B):
                    ps = pps.tile([128, 8, 56], BF16, tag="ps")
                    for ri in range(2):
                        for g in range(4):
                            lg = min(8, FW - g*8)
                            src = Z3[:, ri, cb, g*8:g*8+lg, :].rearrange(
                                "p f t -> p (f t)")
                            nc.tensor.transpose(ps[0:lg*16, ri*4+g, :], src,
                                                ident[0:56, 0:56])
                    for ri in range(2):
                        nc.vector.tensor_copy(
                            Z4[:, ri, cb, 0:3, :], ps[:, ri*4:ri*4+3, :])
                        nc.vector.tensor_copy(
                            Z4[0:80, ri, cb, 3, :], ps[0:80, ri*4+3, :])

            # --- S3: T-fwd -> Z5 [128 | ri2, (c b g fh)]
            pZ3.release()
            pZ5 = tc.alloc_tile_pool(name=f"Z5_{conv}", bufs=1, side="left")
            Z5 = pZ5.tile([128, 2, NC3], BF16)
            z4f = Z4[:].rearrange("p r cb g f -> p r (cb g f)")
            with tc.tile_pool(name="s3ps", bufs=2, space="PSUM") as pps:
                for ch in range(NC3 // 512):
                    re = z4f[:, 0, ch*512:(ch+1)*512]
                    im = z4f[:, 1, ch*512:(ch+1)*512]
                    psr = pps.tile([128, 512], F32, tag="r")
                    psi = pps.tile([128, 512], F32, tag="i")
                    nc.tensor.matmul(psr[:], M("Ctb"), re, start=True, stop=False)
                    nc.tensor.matmul(psr[:], M("Stb"), im, start=False, stop=True)
                    nc.tensor.matmul(psi[:], M("Sntb"), re, start=True, stop=False)
                    nc.tensor.matmul(psi[:], M("Ctb"), im, start=False, stop=True)
                    nc.scalar.activation(Z5[:, 0, ch*512:(ch+1)*512], psr[:], AF.Copy)
                    nc.scalar.activation(Z5[:, 1, ch*512:(ch+1)*512], psi[:], AF.Copy)

            # --- S4: freq multiply -> Z6
            pZ4.release()
            pZ6 = tc.alloc_tile_pool(name=f"Z6_{conv}", bufs=1, side="right")
            Z6 = pZ6.tile([128, 2, NC3], BF16)
            pkf = tc.alloc_tile_pool(name=f"kf{conv}", bufs=1, side="right")
            kfr = pkf.tile([128, NC3], BF16, tag="kfr")
            kfi = pkf.tile([128, NC3], BF16, tag="kfi")
            m1 = pkf.tile([128, NC3], BF16, tag="m1")
            m2 = pkf.tile([128, NC3], BF16, tag="m2")
            nc.sync.dma_start(kfr[:], kf_d[conv, 0])
            nc.sync.dma_start(kfi[:], kf_d[conv, 1])
            nc.vector.tensor_tensor(m1[:], Z5[:, 0], kfr[:], OP.mult)
            nc.vector.tensor_tensor(m2[:], Z5[:, 1], kfi[:], OP.mult)
            nc.vector.tensor_tensor(Z6[:, 0], m1[:], m2[:], OP.subtract)
            nc.vector.tensor_tensor(m1[:], Z5[:, 0], kfi[:], OP.mult)
            nc.vector.tensor_tensor(m2[:], Z5[:, 1], kfr[:], OP.mult)
            nc.vector.tensor_tensor(Z6[:, 1], m1[:], m2[:], OP.add)
            pkf.release()

            # --- S5: T-inv -> Z7
            pZ5.release()
            pZ7 = tc.alloc_tile_pool(name=f"Z7_{conv}", bufs=1, side="left")
            Z7 = pZ7.tile([128, 2, NC3], BF16)
            with tc.tile_pool(name="s5ps", bufs=2, space="PSUM") as pps:
                for ch in range(NC3 // 512):
                    re = Z6[:, 0, ch*512:(ch+1)*512]
                    im = Z6[:, 1, ch*512:(ch+1)*512]
                    psr = pps.tile([128, 512], F32, tag="r")
                    psi = pps.tile([128, 512], F32, tag="i")
                    nc.tensor.matmul(psr[:], M("Ctbi"), re, start=True, stop=False)
                    nc.tensor.matmul(psr[:], M("Sntbi"), im, start=False, stop=True)
                    nc.tensor.matmul(psi[:], M("Stbi"), re, start=True, stop=False)
                    nc.tensor.matmul(psi[:], M("Ctbi"), im, start=False, stop=True)
                    nc.scalar.activation(Z7[:, 0, ch*512:(ch+1)*512], psr[:], AF.Copy)
                    nc.scalar.activation(Z7[:, 1, ch*512:(ch+1)*512], psi[:], AF.Copy)

            # --- R3 -> Z8 [56=fh | ri2, (c b), (g l)29, t16]
            pZ6.release()
            pZ8 = tc.alloc_tile_pool(name=f"Z8_{conv}", bufs=1, side="right")
            Z8 = pZ8.tile([56, 2, S * B, FW, T], BF16)
            z7v = Z7[:].rearrange("p r (cb g f) -> p r cb g f", g=4, f=H)
            with tc.tile_pool(name="r3ps", bufs=2, space="PSUM") as pps:
                for cb in range(S * B):
                    ps = pps.tile([56, 8, 128], BF16, tag="ps")
                    for ri in range(2):
                        for g in range(4):
                            lg = min(8, FW - g*8)
                            nc.tensor.transpose(
                                ps[:, ri*4+g, 0:lg*16], z7v[0:lg*16, ri, cb, g, :],
                                ident[0:lg*16, 0:lg*16])
                    for ri in range(2):
                        nc.scalar.activation(
                            Z8[:, ri, cb].rearrange("p q t -> p (q t)"),
                            ps[:, ri*4:(ri+1)*4, :].rearrange(
                                "p g n -> p (g n)")[:, 0:FW*T], AF.Copy)

            # --- S6: H-inv -> Z9p [56=h | ri2, (c b), q32, t16]
            pZ7.release()
            pZ9p = tc.alloc_tile_pool(name=f"Z9p_{conv}", bufs=1, side="left")
            Z9p = pZ9p.tile([56, S * B, 2, 32, T], BF16)
            nc.vector.memset(
                Z9p[:].rearrange("p cb r q t -> p (cb r) q t")[:, :, 29:32, :], 0.0)
            with tc.tile_pool(name="s6ps", bufs=2, space="PSUM") as pps:
                for cb in range(S * B):
                    re = Z8[:, 0, cb].rearrange("p q t -> p (q t)")
                    im = Z8[:, 1, cb].rearrange("p q t -> p (q t)")
                    psr = pps.tile([56, 464], F32, tag="r")
                    psi = pps.tile([56, 464], F32, tag="i")
                    nc.tensor.matmul(psr[:], M("CHi")[0:56], re, start=True, stop=False)
                    nc.tensor.matmul(psr[:], M("SnHi")[0:56], im, start=False, stop=True)
                    nc.tensor.matmul(psi[:], M("SHi")[0:56], re, start=True, stop=False)
                    nc.tensor.matmul(psi[:], M("CHi")[0:56], im, start=False, stop=True)
                    for ri, psx in ((0, psr), (1, psi)):
                        nc.scalar.activation(
                            Z9p[:, cb, ri, 0:FW, :].rearrange("p q t -> p (q t)"),
                            psx[:], AF.Copy)

            # --- R4 -> Z9 [64=(ri2,q32) | (c b), t16, h56]
            pZ8.release()
            pZ9 = tc.alloc_tile_pool(name=f"Z9_{conv}", bufs=1, side="right")
            Z9 = pZ9.tile([64, S * B, T, H], BF16)

            with tc.tile_pool(name="r4ps", bufs=2, space="PSUM") as pps:
                for cb in range(S * B):
                    ps = pps.tile([64, T, 56], BF16, tag="ps")
                    for t_ in range(T):
                        nc.tensor.transpose(
                            ps[:, t_, :],
                            Z9p[:, cb, :, :, t_].rearrange("p r q -> p (r q)"),
                            ident[0:56, 0:56])
                    nc.vector.tensor_copy(
                        Z9[:, cb].rearrange("p t h -> p (t h)"),
                        ps[:].rearrange("p t h -> p (t h)"))

            # --- S7: W-inv -> Gsp chunks -> dram
            pZ9p.release()
            pgs = tc.alloc_tile_pool(name=f"gsp{conv}", bufs=3, side="left")
            with tc.tile_pool(name="s7ps", bufs=2, space="PSUM") as pps:
                for cb in range(S * B):
                    gsp = pgs.tile([W, H, T], BF16, tag="gsp")
                    for th2 in range(2):
                        ps = pps.tile([56, 448], F32, tag="ps")
                        nc.tensor.matmul(
                            ps[:], M("CWst")[0:64],
                            Z9[:, cb, th2*8:(th2+1)*8, :].rearrange(
                                "p t h -> p (t h)"),
                            start=True, stop=True)
                        nc.scalar.activation(
                            gsp[:, :, th2*8:(th2+1)*8].rearrange("p h t -> p t h"),
                            ps[:].rearrange("p (t h) -> p t h", t=8), AF.Copy)
                    nc.sync.dma_start(
                        Gsp_d[conv].rearrange("w (cb n) -> w cb n", cb=S*B)[:, cb, :],
                        gsp[:].rearrange("p h t -> p (h t)"))
            pgs.release()
            pZ9.release()
            pin.release()

            # readback in scan layout [128=(c,b,hg) | (w, hl, t)]
            dst_sc = g_sc if conv == 0 else h_sc
            nc.sync.dma_start(
                dst_sc[:].rearrange("p (w n) -> p w n", w=W),
                Gsp_d[conv].rearrange("w (p n) -> p w n", p=128))

        # ---------------- Scan ----------------
        with tc.tile_pool(name="scan", bufs=1) as psn:
            Ft = psn.tile([128, NTOK], BF16)
            Vt = psn.tile([128, NTOK], BF16)
            Cs = psn.tile([128, NTOK], BF16)
            nc.scalar.activation(Ft[:], g_sc[:], AF.Sigmoid, scale=-1.0)
            nc.vector.memset(
                Ft[:].rearrange("p (n t) -> p n t", t=T)[:, :, 0:1], 0.0)
            nc.scalar.activation(Vt[:], g_sc[:], AF.Sigmoid)
            nc.vector.tensor_tensor(Cs[:], h_sc[:], h_sc[:], OP.mult)
            nc.vector.tensor_tensor(Vt[:], Vt[:], Cs[:], OP.mult)
            nc.vector.tensor_tensor_scan(Cs[:], Ft[:], Vt[:], 0.0, OP.mult, OP.add)
            nc.sync.dma_start(
                a2a_in_d[:].rearrange("j s n -> s j n"), Cs[:])
        psc.release()

        nc.gpsimd.collective_compute(
            "AllToAll", OP.bypass,
            ins=[a2a_in_d[:].opt()],
            outs=[a2a_out_d[:].opt()],
            replica_groups=[list(range(NCORES))],
        )

        # ---------------- Phase 3 ----------------
        with (
            tc.tile_pool(name="p3", bufs=1) as p3,
            tc.tile_pool(name="p3c", bufs=3) as p3c,
        ):
            Cr = p3.tile([C, NTOK], BF16)
            nc.sync.dma_start(Cr[:], a2a_out_d[:])
            wo = p3.tile([C, C], BF16)
            nc.sync.dma_start(wo[:], wo_d[:])
            w1 = p3.tile([C, HID], BF16)
            nc.sync.dma_start(w1[:], w1_d[:])
            w2 = p3.tile([C, HID], BF16)
            nc.sync.dma_start(w2[:], w2l_d[:])
            onek = p3.tile([128, 1], BF16)
            nc.vector.memset(onek[:], 1.0 / 128.0)
            one1 = p3.tile([1, 128], BF16)
            nc.vector.memset(one1[:], 1.0)
            epsb = p3.tile([1, 1], F32)
            nc.vector.memset(epsb[:], 1e-6)

            ssm = p3.tile([C, NTOK], BF16)
            MB = p3.tile([C, NTOK], BF16)
            RB = p3.tile([C, NTOK], BF16)
            NCH = NTOK // 448
            with (
                tc.tile_pool(name="p3ps1", bufs=2, space="PSUM") as pA,
                tc.tile_pool(name="p3ps2", bufs=1, space="PSUM") as pB,
            ):
                for ch in range(NCH):
                    sl = slice(ch*448, (ch+1)*448)
                    ps = pA.tile([C, 448], F32, tag="wo")
                    nc.tensor.matmul(ps[:], wo[:], Cr[:, sl], start=True, stop=True)
                    nc.scalar.activation(ssm[:, sl], ps[:], AF.Identity, bias=bia[:, 1:2])
                    sq = p3c.tile([C, 448], BF16, tag="sq")
                    nc.scalar.activation(sq[:], ssm[:, sl], AF.Square)
                    psm = pB.tile([1, 448], F32, tag="m")
                    psq = pB.tile([1, 448], F32, tag="q")
                    nc.tensor.matmul(psm[:], onek[:], ssm[:, sl], start=True, stop=True)
                    nc.tensor.matmul(psq[:], onek[:], sq[:], start=True, stop=True)
                    mr = p3c.tile([1, 448], BF16, tag="mr")
                    nc.vector.tensor_copy(mr[:], psm[:])
                    v1 = p3c.tile([1, 448], F32, tag="v1")
                    nc.vector.tensor_tensor(v1[:], psm[:], mr[:], OP.mult)
                    nc.vector.tensor_tensor(v1[:], psq[:], v1[:], OP.subtract)
                    sd = p3c.tile([1, 448], F32, tag="sd")
                    nc.scalar.activation(sd[:], v1[:], AF.Sqrt, bias=epsb[:])
                    rsf = p3c.tile([1, 448], F32, tag="rsf")
                    nc.vector.reciprocal(rsf[:], sd[:])
                    rs = p3c.tile([1, 448], BF16, tag="rs")
                    nc.vector.tensor_copy(rs[:], rsf[:])
                    psbm = pB.tile([128, 448], F32, tag="bm")
                    psbr = pB.tile([128, 448], F32, tag="br")
                    nc.tensor.matmul(psbm[:], one1[:], mr[:], start=True, stop=True)
                    nc.tensor.matmul(psbr[:], one1[:], rs[:], start=True, stop=True)
                    nc.vector.tensor_copy(MB[:, sl], psbm[:])
                    nc.vector.tensor_copy(RB[:, sl], psbr[:])

            yt = p3.tile([C, NTOK], BF16)
            with tc.tile_pool(name="p3ps3", bufs=2, space="PSUM") as pC:
                for ch in range(NCH):
                    sl = slice(ch*448, (ch+1)*448)
                    d = p3c.tile([C, 448], BF16, tag="xn1")
                    nc.vector.tensor_tensor(d[:], ssm[:, sl], MB[:, sl], OP.subtract)
                    nc.vector.tensor_tensor(d[:], d[:], RB[:, sl], OP.mult)
                    xn = p3c.tile([C, 448], BF16, tag="xn2")
                    nc.vector.tensor_scalar(
                        out=xn[:], in0=d[:], scalar1=bia[:, 7:8],
                        scalar2=bia[:, 8:9], op0=OP.mult, op1=OP.add)
                    hts = p3c.tile([C, 4, 448], BF16, tag="ht")
                    hp = p3c.tile([C, 4, 448], BF16, tag="hp")
                    for j in range(4):
                        psh = pC.tile([C, 448], F32, tag="m1")
                        nc.tensor.matmul(psh[:], w1[:, j*128:(j+1)*128], xn[:],
                                         start=True, stop=True)
                        nc.scalar.activation(hp[:, j, :], psh[:],
                                             AF.Identity, bias=bia[:, 2+j:3+j])
                        nc.scalar.activation(hts[:, j, :], psh[:], AF.Sigmoid,
                                             bias=bia[:, 9+j:10+j], scale=1.702)
                    nc.vector.tensor_tensor(
                        hts[:].rearrange("p j n -> p (j n)"),
                        hts[:].rearrange("p j n -> p (j n)"),
                        hp[:].rearrange("p j n -> p (j n)"), OP.mult)
                    psy = pC.tile([C, 448], F32, tag="m2")
                    for j in range(4):
                        nc.tensor.matmul(psy[:], w2[:, j*128:(j+1)*128], hts[:, j, :],
                                         start=(j == 0), stop=(j == 3))
                    nc.scalar.activation(yt[:, sl], psy[:], AF.Identity, bias=bia[:, 6:7])

            NTC = NTOK // 128
            with (
                tc.tile_pool(name="p3ps4", bufs=2, space="PSUM") as pD,
                tc.tile_pool(name="p3x", bufs=2) as p3x,
            ):
                for k0 in range(0, NTC, 7):
                    nn_ = min(7, NTC - k0)
                    xrs = p3x.tile([128, 7, 128], F32, tag="xrs")
                    nc.sync.dma_start(
                        xrs[:, 0:nn_, :],
                        xres_d[:].rearrange("(n p) c -> p n c", p=128)[:, k0:k0+nn_, :])
                    yo = p3x.tile([128, 7, 128], F32, tag="yo")
                    for q in range(nn_):
                        ch = k0 + q
                        pst = pD.tile([128, 128], BF16, tag="t")
                        nc.tensor.transpose(pst[:], yt[:, ch*128:(ch+1)*128], ident[:])
                        nc.vector.tensor_tensor(yo[:, q, :], pst[:], xrs[:, q, :],
                                                OP.add)
                    nc.sync.dma_start(
                        y_d[:].rearrange("(n p) c -> p n c", p=128)[:, k0:k0+nn_, :],
                        yo[:, 0:nn_, :])
    _split_multi_waits(nc)
    return nc


def _split_multi_waits(nc):
    """TRN2 TPB instructions carry exactly one sync wait; hoist extras
    onto preceding same-engine NoOps."""
    n = [0]
    for f in nc.m.functions:
        for blk in f.blocks:
            insts = blk.instructions
            i = 0
            while i < len(insts):
                inst = insts[i]
                si = inst.sync_info
                if si is not None and len(si.on_wait) > 1:
                    waits = list(si.on_wait)
                    for w in waits[:-1]:
                        nop = mybir.InstNoOp(
                            name=f"I-wsplit-{n[0]}", ins=[], outs=[])
                        n[0] += 1
                        nop.engine = inst.engine
                        nop.sync_info = mybir.SyncInfo(
                            on_wait=[w], on_update=[])
                        nc.register_instruction(nop)
                        insts.insert(i, nop)
                        i += 1
                    si.on_wait = [waits[-1]]
                i += 1


# ---------------- entry point ----------------
_NC_CACHE = [None]
LAST_RESULTS = None


def kernel(**inputs):
    global LAST_RESULTS
    import concourse.bass_utils as bass_utils
    if _NC_CACHE[0] is None:
        _NC_CACHE[0] = build_nc()
    nc = _NC_CACHE[0]
    in_maps = host_prep(inputs)
    res = bass_utils.run_bass_kernel_spmd(
        nc, in_maps, core_ids=list(range(NCORES)))
    LAST_RESULTS = res
    return assemble_output(res.results)


# revision 5
# speedup vs baseline: 3.4989x; 3.4989x over previous
"""Device-kernel builder + host prep for nn_GatedConvNeXt3DBlock on 8 trn2 cores.

Sharding: channels (16/core) for the FFT conv + scan; one AllToAll to
token blocks (b, h-band) for Wo/LN/MLP/residual.
"""
import numpy as np
import ml_dtypes

import concourse.bass as bass
import concourse.tile as tile
import concourse.mybir as mybir
from concourse.masks import make_identity

F32 = mybir.dt.float32
BF16 = mybir.dt.bfloat16
AF = mybir.ActivationFunctionType
OP = mybir.AluOpType
BF = ml_dtypes.bfloat16

B, T, H, W, C = 2, 16, 56, 56, 128
HID = 4 * C
S = 16            # channels per core
NCORES = 8
N = B * T * H * W          # 100352 tokens
FW = 29                    # rfft bins along W
NTOK = N // NCORES         # 12544 tokens per core in phase 3
HB = 14                    # h rows per (b, hg) block
NC3 = S * B * 4 * H        # 7168 free cols in T-stage layout

_OFF = {}

def _build_dft_cols():
    cols = []
    off = 0

    def put(name, mat, rows0=0, dup64=False):
        nonlocal off
        a = np.zeros((128, mat.shape[1]), np.float32)
        a[rows0:rows0 + mat.shape[0]] = mat
        if dup64:
            a[64:64 + mat.shape[0]] = mat
        _OFF[name] = (off, mat.shape[1])
        cols.append(a)
        off += mat.shape[1]

    w = np.arange(W); f = np.arange(FW)
    th = 2 * np.pi * np.outer(w, f) / W
    put("A1", np.concatenate([np.cos(th), -np.sin(th)], axis=1))  # [56, 58]

    h = np.arange(H)
    thh = 2 * np.pi * np.outer(h, h) / H
    CH, SH = np.cos(thh), np.sin(thh)
    for nm, m in [("CH", CH), ("SH", SH), ("SnH", -SH),
                  ("CHi", CH / H), ("SHi", SH / H), ("SnHi", -SH / H)]:
        put(nm, m, dup64=True)

    t = np.arange(T)
    tht = 2 * np.pi * np.outer(t, t) / T
    Ct, St = np.cos(tht), np.sin(tht)

    def bd(m):
        out = np.zeros((128, 128), np.float32)
        for l in range(8):
            out[l*16:(l+1)*16, l*16:(l+1)*16] = m
        return out

    for nm, m in [("Ctb", bd(Ct)), ("Stb", bd(St)), ("Sntb", bd(-St)),
                  ("Ctbi", bd(Ct / T)), ("Stbi", bd(St / T)), ("Sntbi", bd(-St / T))]:
        put(nm, m)

    a_f = np.full(FW, 2.0); a_f[0] = 1.0; a_f[28] = 1.0
    thw = 2 * np.pi * np.outer(f, w) / W
    m = np.zeros((64, W), np.float32)
    m[0:29] = (a_f[:, None] * np.cos(thw)) / W
    m[32:61] = (-a_f[:, None] * np.sin(thw)) / W
    put("CWst", m)
    return np.concatenate(cols, axis=1)

DFTM = _build_dft_cols()
DFT_COLS = DFTM.shape[1]


def _kernel_to_freq(kernel):
    c, kt, kh, kw = kernel.shape
    ti = (np.arange(kt) - kt // 2) % T
    hi = (np.arange(kh) - kh // 2) % H
    wi = (np.arange(kw) - kw // 2) % W
    padded = np.zeros((c, T, H, W), np.float32)
    tt, th_, tw = np.meshgrid(ti, hi, wi, indexing="ij")
    padded[:, tt, th_, tw] = kernel
    return np.fft.fftn(padded, axes=(1, 2, 3))


def host_prep(inputs):
    x = np.asarray(inputs["x"], np.float32)
    Wg = np.asarray(inputs["Wg"], np.float32); bg = np.asarray(inputs["bg"], np.float32)
    Wh = np.asarray(inputs["Wh"], np.float32); bh = np.asarray(inputs["bh"], np.float32)
    Wo = np.asarray(inputs["Wo"], np.float32); bo = np.asarray(inputs["bo"], np.float32)
    W1 = np.asarray(inputs["W1"], np.float32); b1 = np.asarray(inputs["b1"], np.float32)
    W2 = np.asarray(inputs["W2"], np.float32); b2 = np.asarray(inputs["b2"], np.float32)
    gamma = np.asarray(inputs["gamma"], np.float32)
    ln_s = np.asarray(inputs["ln_scale"], np.float32)
    ln_b = np.asarray(inputs["ln_bias"], np.float32)

    xt = np.ascontiguousarray(x.reshape(N, C).T).astype(BF)

    gkf = _kernel_to_freq(np.asarray(inputs["gate_kernel"], np.float32))[:, :, :, :FW]
    hkf = _kernel_to_freq(np.asarray(inputs["hidden_kernel"], np.float32))[:, :, :, :FW]

    W2g = W2 * gamma[None, :]
    b2g = b2 * gamma
    w2l = np.concatenate([W2g[j*128:(j+1)*128, :] for j in range(4)], axis=1)
    dftm = DFTM.astype(BF)

    in_maps = []
    for i in range(NCORES):
        cs = slice(S * i, S * (i + 1))
        wgh = np.concatenate([Wg[:, cs], Wh[:, cs]], axis=1).astype(BF)
        bgh = np.concatenate([bg[cs], bh[cs]])

        # kf[conv, ri, p=(l*16+ft), (c, b, g, fh)]
        kf = np.zeros((2, 2, 128, S, B, 4, H), np.float32)
        for ci, kfull in enumerate((gkf, hkf)):
            sub = kfull[cs]                       # [S, T, H, FW]
            for l in range(8):
                for g in range(4):
                    fw = g * 8 + l
                    if fw >= FW:
                        continue
                    v = sub[:, :, :, fw]          # [S, T, H]
                    for ri, vv in ((0, v.real), (1, v.imag)):
                        blk = vv.transpose(1, 0, 2).astype(np.float32)  # [T, S, H]
                        for ft in range(T):
                            kf[ci, ri, l*16+ft, :, 0, g, :] = blk[ft]
                            kf[ci, ri, l*16+ft, :, 1, g, :] = blk[ft]
        kf = np.ascontiguousarray(kf.reshape(2, 2, 128, NC3)).astype(BF)

        b_i, hg = i // 4, i % 4
        xslab = x[b_i, :, HB*hg:HB*(hg+1), :, :]
        xres = np.ascontiguousarray(
            xslab.transpose(2, 1, 0, 3).reshape(NTOK, C)).astype(np.float32)

        biases = np.zeros((128, 13), np.float32)
        biases[0:32, 0] = bgh
        biases[:, 1] = bo
        biases[:, 2:6] = b1.reshape(4, 128).T
        biases[:, 6] = b2g
        biases[:, 7] = ln_s
        biases[:, 8] = ln_b
        biases[:, 9:13] = 1.702 * biases[:, 2:6]

        in_maps.append({
            "xt": xt, "xres": xres, "wgh": wgh, "kf": kf,
            "wo": Wo.astype(BF), "w1": W1.astype(BF), "w2l": w2l.astype(BF),
            "dftm": dftm, "biases": biases,
        })
    return in_maps


def assemble_output(results):
    out = np.empty((B, T, H, W, C), np.float32)
    for i in range(NCORES):
        y = np.asarray(results[i]["y"])
        b_i, hg = i // 4, i % 4
        out[b_i, :, HB*hg:HB*(hg+1), :, :] = (
            y.reshape(W, HB, T, C).transpose(2, 1, 0, 3))
    return out


def build_nc():
    nc = bass.Bass()
    xt_d = nc.dram_tensor("xt", [C, N], BF16, kind="ExternalInput")
    xres_d = nc.dram_tensor("xres", [NTOK, C], F32, kind="ExternalInput")
    wgh_d = nc.dram_tensor("wgh", [C, 32], BF16, kind="ExternalInput")
    kf_d = nc.dram_tensor("kf", [2, 2, 128, NC3], BF16, kind="ExternalInput")
    wo_d = nc.dram_tensor("wo", [C, C], BF16, kind="ExternalInput")
    w1_d = nc.dram_tensor("w1", [C, HID], BF16, kind="ExternalInput")
    w2l_d = nc.dram_tensor("w2l", [C, HID], BF16, kind="ExternalInput")
    dftm_d = nc.dram_tensor("dftm", [128, DFT_COLS], BF16, kind="ExternalInput")
    bias_d = nc.dram_tensor("biases", [128, 13], F32, kind="ExternalInput")
    y_d = nc.dram_tensor("y", [NTOK, C], F32, kind="ExternalOutput")

    G_d = nc.dram_tensor("Gs", [32, N], BF16)
    Gsp_d = nc.dram_tensor("Gsp", [2, W, S * B * H * T], BF16)
    a2a_in_d = nc.dram_tensor("a2a_in", [NCORES, S, NTOK], BF16)
    a2a_out_d = nc.dram_tensor("a2a_out", [C, NTOK], BF16)

    from contextlib import ExitStack
    with tile.TileContext(nc) as tc, ExitStack() as _ctx:
        consts = _ctx.enter_context(tc.tile_pool(name="consts", bufs=1))
        dft = consts.tile([128, DFT_COLS], BF16)
        nc.sync.dma_start(dft[:], dftm_d[:])
        bia = consts.tile([128, 13], F32)
        nc.sync.dma_start(bia[:], bias_d[:])
        wgh = consts.tile([C, 32], BF16)
        nc.sync.dma_start(wgh[:], wgh_d[:])
        ident = consts.tile([128, 128], BF16)
        make_identity(nc, ident[:])

        def M(name):
            o, n_ = _OFF[name]
            return dft[:, o:o + n_]

        # ---------------- Phase 1 ----------------
        with (
            tc.tile_pool(name="p1x", bufs=3) as p1x,
            tc.tile_pool(name="p1g", bufs=3) as p1g,
            tc.tile_pool(name="p1ps", bufs=2, space="PSUM") as p1ps,
        ):
            CHT = 2048
            for k in range(N // CHT):
                xtile = p1x.tile([C, CHT], BF16, tag="xt")
                nc.sync.dma_start(xtile[:], xt_d[:, k*CHT:(k+1)*CHT])
                ps = p1ps.tile([32, CHT], F32, tag="ps")
                for s_ in range(4):
                    nc.tensor.matmul(ps[:, s_*512:(s_+1)*512], wgh[:],
                                     xtile[:, s_*512:(s_+1)*512],
                                     start=True, stop=True)
                gt = p1g.tile([32, CHT], BF16, tag="gt")
                for s_ in range(4):
                    nc.scalar.activation(gt[:, s_*512:(s_+1)*512],
                                         ps[:, s_*512:(s_+1)*512],
                                         AF.Identity, bias=bia[0:32, 0:1])
                nc.sync.dma_start(G_d[:, k*CHT:(k+1)*CHT], gt[:])

        # ---------------- Phase 2 ----------------
        # pool discipline: odd stages on left stack, even on right; LIFO per side
        psc = tc.alloc_tile_pool(name="scbuf", bufs=1, side="left")
        g_sc = psc.tile([128, NTOK], BF16, tag="gsc")
        h_sc = psc.tile([128, NTOK], BF16, tag="hsc")

        for conv in range(2):
            pin = tc.alloc_tile_pool(name=f"cin{conv}", bufs=3, side="left")

            # --- S1: W-fwd -> Z1 [58 | c,b,(t h)]
            pZ1 = tc.alloc_tile_pool(name=f"Z1_{conv}", bufs=1, side="left")
            Z1 = pZ1.tile([58, S, B, T * H], BF16)
            with (
                tc.tile_pool(name="s1ps", bufs=2, space="PSUM") as pps,
                tc.tile_pool(name="s1tp", bufs=2, space="PSUM") as ptp,
            ):
                for c in range(S):
                    for b in range(B):
                        xn_ = pin.tile([128, 7, W], BF16, tag="xn")
                        src = G_d[conv*16 + c, b*T*H*W:(b+1)*T*H*W].rearrange(
                            "(k p w) -> p k w", p=128, w=W)
                        nc.sync.dma_start(xn_[:], src)
                        pt = ptp.tile([W, 7, 128], BF16, tag="pt")
                        for k in range(7):
                            nc.tensor.transpose(pt[:, k, :], xn_[:, k, :],
                                                ident[:])
                        xw = pin.tile([W, T * H], BF16, tag="xw")
                        nc.vector.tensor_copy(
                            xw[:], pt[:].rearrange("p k n -> p (k n)"))
                        ps = pps.tile([58, 1024], F32, tag="ps")
                        nc.tensor.matmul(ps[:, 0:448], M("A1")[0:56], xw[:, 0:448],
                                         start=True, stop=True)
                        nc.tensor.matmul(ps[:, 512:960], M("A1")[0:56], xw[:, 448:896],
                                         start=True, stop=True)
                        nc.scalar.activation(
                            Z1[:, c, b, 0:448], ps[:, 0:448], AF.Copy)
                        nc.scalar.activation(
                            Z1[:, c, b, 448:896], ps[:, 512:960], AF.Copy)

            # --- R1 -> Z2 [120=(t-par,h) | c,b,tp8,58]
            pZ2 = tc.alloc_tile_pool(name=f"Z2_{conv}", bufs=1, side="right")
            Z2 = pZ2.tile([120, 2, S * B * 8, FW], BF16)
            with tc.tile_pool(name="r1ps", bufs=2, space="PSUM") as pps:
                for c in range(S):
                    for b in range(B):
                        ps = pps.tile([120, 8, 58], BF16, tag="ps")
                        for tp in range(8):
                            nc.tensor.transpose(
                                ps[0:56, tp, :], Z1[:, c, b, (2*tp)*H:(2*tp+1)*H],
                                ident[0:58, 0:58])
                            nc.tensor.transpose(
                                ps[64:120, tp, :], Z1[:, c, b, (2*tp+1)*H:(2*tp+2)*H],
                                ident[0:58, 0:58])
                        srcv = ps[:].rearrange("p t (r f) -> p t r f", r=2)
                        cb0 = (c * B + b) * 8
                        dstv = Z2[:].rearrange("p r x f -> p x r f")[:, cb0:cb0+8]
                        nc.vector.tensor_copy(dstv[0:56], srcv[0:56])
                        nc.vector.tensor_copy(dstv[64:120], srcv[64:120])

            # --- S2: H-fwd -> Z3 [56 | ri2, c,b,t16,fw29]
            pZ1.release()
            pZ3 = tc.alloc_tile_pool(name=f"Z3_{conv}", bufs=1, side="left")
            Z3 = pZ3.tile([56, 2, S * B, FW, T], BF16)
            with tc.tile_pool(name="s2ps", bufs=2, space="PSUM") as pps:
                for par in range(2):
                    pb = 0 if par == 0 else 64
                    for ch in range(16):
                        re = Z2[pb:pb+56, 0, ch*16:(ch+1)*16, :]
                        im = Z2[pb:pb+56, 1, ch*16:(ch+1)*16, :]
                        psr = pps.tile([56, 512], F32, tag="r")
                        psi = pps.tile([56, 512], F32, tag="i")
                        nc.tensor.matmul(psr[:, 0:464], M("CH")[pb:pb+56], re,
                                         start=True, stop=False)
                        nc.tensor.matmul(psr[:, 0:464], M("SH")[pb:pb+56], im,
                                         start=False, stop=True)
                        nc.tensor.matmul(psi[:, 0:464], M("SnH")[pb:pb+56], re,
                                         start=True, stop=False)
                        nc.tensor.matmul(psi[:, 0:464], M("CH")[pb:pb+56], im,
                                         start=False, stop=True)
                        for ri, psx in ((0, psr), (1, psi)):
                            dst = Z3[:, ri, ch*2:(ch+1)*2, :, par::2]
                            nc.scalar.activation(
                                dst.rearrange("p cb f t -> p cb t f"),
                                psx[:, 0:464].rearrange(
                                    "p (cb t f) -> p cb t f", cb=2, t=8), AF.Copy)

            # --- R2 -> Z4 [128=(l8,t16) | ri2, (c b), g4, fh56]
            pZ2.release()
            pZ4 = tc.alloc_tile_pool(name=f"Z4_{conv}", bufs=1, side="right")
            Z4 = pZ4.tile([128, 2, S * B, 4, H], BF16)
            nc.vector.memset(
                Z4[:].rearrange("p r cb g f -> p (r cb) g f")[64:128, :, 3, :], 0.0)

            with tc.tile_pool(name="r2ps", bufs=2, space="PSUM") as pps:
                for cb in range(S * B):
                    ps = pps.tile([128, 8, 56], BF16, tag="ps")
                    for ri in range(2):
                        for g in range(4):
                            lg = min(8, FW - g*8)
                            src = Z3[:, ri, cb, g*8:g*8+lg, :].rearrange(
                                "p f t -> p (f t)")
                            nc.tensor.transpose(ps[0:lg*16, ri*4+g, :], src,
                                                ident[0:56, 0:56])
                    for ri in range(2):
                        nc.vector.tensor_copy(
                            Z4[:, ri, cb, 0:3, :], ps[:, ri*4:ri*4+3, :])
                        nc.vector.tensor_copy(
                            Z4[0:80, ri, cb, 3, :], ps[0:80, ri*4+3, :])

            # --- S3: T-fwd -> Z5 [128 | ri2, (c b g fh)]
            pZ3.release()
            pZ5 = tc.alloc_tile_pool(name=f"Z5_{conv}", bufs=1, side="left")
            Z5 = pZ5.tile([128, 2, NC3], BF16)
            z4f = Z4[:].rearrange("p r cb g f -> p r (cb g f)")
            with tc.tile_pool(name="s3ps", bufs=2, space="PSUM") as pps:
                for ch in range(NC3 // 512):
                    re = z4f[:, 0, ch*512:(ch+1)*512]
                    im = z4f[:, 1, ch*512:(ch+1)*512]
                    psr = pps.tile([128, 512], F32, tag="r")
                    psi = pps.tile([128, 512], F32, tag="i")
                    nc.tensor.matmul(psr[:], M("Ctb"), re, start=True, stop=False)
                    nc.tensor.matmul(psr[:], M("Stb"), im, start=False, stop=True)
                    nc.tensor.matmul(psi[:], M("Sntb"), re, start=True, stop=False)
                    nc.tensor.matmul(psi[:], M("Ctb"), im, start=False, stop=True)
                    nc.scalar.activation(Z5[:, 0, ch*512:(ch+1)*512], psr[:], AF.Copy)
                    nc.scalar.activation(Z5[:, 1, ch*512:(ch+1)*512], psi[:], AF.Copy)

            # --- S4: freq multiply -> Z6
            pZ4.release()
            pZ6 = tc.alloc_tile_pool(name=f"Z6_{conv}", bufs=1, side="right")
            Z6 = pZ6.tile([128, 2, NC3], BF16)
            pkf = tc.alloc_tile_pool(name=f"kf{conv}", bufs=1, side="right")
            kfr = pkf.tile([128, NC3], BF16, tag="kfr")
            kfi = pkf.tile([128, NC3], BF16, tag="kfi")
            m1 = pkf.tile([128, NC3], BF16, tag="m1")
            m2 = pkf.tile([128, NC3], BF16, tag="m2")
            nc.sync.dma_start(kfr[:], kf_d[conv, 0])
            nc.sync.dma_start(kfi[:], kf_d[conv, 1])
            nc.vector.tensor_tensor(m1[:], Z5[:, 0], kfr[:], OP.mult)
            nc.vector.tensor_tensor(m2[:], Z5[:, 1], kfi[:], OP.mult)
            nc.vector.tensor_tensor(Z6[:, 0], m1[:], m2[:], OP.subtract)
            nc.vector.tensor_tensor(m1[:], Z5[:, 0], kfi[:], OP.mult)
            nc.vector.tensor_tensor(m2[:], Z5[:, 1], kfr[:], OP.mult)
            nc.vector.tensor_tensor(Z6[:, 1], m1[:], m2[:], OP.add)
            pkf.release()

            # --- S5: T-inv -> Z7
            pZ5.release()
            pZ7 = tc.alloc_tile_pool(name=f"Z7_{conv}", bufs=1, side="left")
            Z7 = pZ7.tile([128, 2, NC3], BF16)
            with tc.tile_pool(name="s5ps", bufs=2, space="PSUM") as pps:
                for ch in range(NC3 // 512):
                    re = Z6[:, 0, ch*512:(ch+1)*512]
                    im = Z6[:, 1, ch*512:(ch+1)*512]
                    psr = pps.tile([128, 512], F32, tag="r")
                    psi = pps.tile([128, 512], F32, tag="i")
                    nc.tensor.matmul(psr[:], M("Ctbi"), re, start=True, stop=False)
                    nc.tensor.matmul(psr[:], M("Sntbi"), im, start=False, stop=True)
                    nc.tensor.matmul(psi[:], M("Stbi"), re, start=True, stop=False)
                    nc.tensor.matmul(psi[:], M("Ctbi"), im, start=False, stop=True)
                    nc.scalar.activation(Z7[:, 0, ch*512:(ch+1)*512], psr[:], AF.Copy)
                    nc.scalar.activation(Z7[:, 1, ch*512:(ch+1)*512], psi[:], AF.Copy)

            # --- R3 -> Z8 [56=fh | ri2, (c b), (g l)29, t16]
            pZ6.release()
            pZ8 = tc.alloc_tile_pool(name=f"Z8_{conv}", bufs=1, side="right")
            Z8 = pZ8.tile([56, 2, S * B, FW, T], BF16)
            z7v = Z7[:].rearrange("p r (cb g f) -> p r cb g f", g=4, f=H)
            with tc.tile_pool(name="r3ps", bufs=2, space="PSUM") as pps:
                for cb in range(S * B):
                    ps = pps.tile([56, 8, 128], BF16, tag="ps")
                    for ri in range(2):
                        for g in range(4):
                            lg = min(8, FW - g*8)
                            nc.tensor.transpose(
                                ps[:, ri*4+g, 0:lg*16], z7v[0:lg*16, ri, cb, g, :],
                                ident[0:lg*16, 0:lg*16])
                    for ri in range(2):
                        nc.scalar.activation(
                            Z8[:, ri, cb].rearrange("p q t -> p (q t)"),
                            ps[:, ri*4:(ri+1)*4, :].rearrange(
                                "p g n -> p (g n)")[:, 0:FW*T], AF.Copy)

            # --- S6: H-inv -> Z9p [56=h | ri2, (c b), q32, t16]
            pZ7.release()
            pZ9p = tc.alloc_tile_pool(name=f"Z9p_{conv}", bufs=1, side="left")
            Z9p = pZ9p.tile([56, S * B, 2, 32, T], BF16)
            nc.vector.memset(
                Z9p[:].rearrange("p cb r q t -> p (cb r) q t")[:, :, 29:32, :], 0.0)
            with tc.tile_pool(name="s6ps", bufs=2, space="PSUM") as pps:
                for cb in range(S * B):
                    re = Z8[:, 0, cb].rearrange("p q t -> p (q t)")
                    im = Z8[:, 1, cb].rearrange("p q t -> p (q t)")
                    psr = pps.tile([56, 464], F32, tag="r")
                    psi = pps.tile([56, 464], F32, tag="i")
                    nc.tensor.matmul(psr[:], M("CHi")[0:56], re, start=True, stop=False)
                    nc.tensor.matmul(psr[:], M("SnHi")[0:56], im, start=False, stop=True)
                    nc.tensor.matmul(psi[:], M("SHi")[0:56], re, start=True, stop=False)
                    nc.tensor.matmul(psi[:], M("CHi")[0:56], im, start=False, stop=True)
                    for ri, psx in ((0, psr), (1, psi)):
                        nc.scalar.activation(
                            Z9p[:, cb, ri, 0:FW, :].rearrange("p q t -> p (q t)"),
                            psx[:], AF.Copy)

            # --- R4 -> Z9 [64=(ri2,q32) | (c b), t16, h56]
            pZ8.release()
            pZ9 = tc.alloc_tile_pool(name=f"Z9_{conv}", bufs=1, side="right")
            Z9 = pZ9.tile([64, S * B, T, H], BF16)

            with tc.tile_pool(name="r4ps", bufs=2, space="PSUM") as pps:
                for cb in range(S * B):
                    ps = pps.tile([64, T, 56], BF16, tag="ps")
                    for t_ in range(T):
                        nc.tensor.transpose(
                            ps[:, t_, :],
                            Z9p[:, cb, :, :, t_].rearrange("p r q -> p (r q)"),
                            ident[0:56, 0:56])
                    nc.vector.tensor_copy(
                        Z9[:, cb].rearrange("p t h -> p (t h)"),
                        ps[:].rearrange("p t h -> p (t h)"))

            # --- S7: W-inv -> Gsp chunks -> dram
            pZ9p.release()
            pgs = tc.alloc_tile_pool(name=f"gsp{conv}", bufs=3, side="left")
            with tc.tile_pool(name="s7ps", bufs=2, space="PSUM") as pps:
                for cb in range(S * B):
                    gsp = pgs.tile([W, H, T], BF16, tag="gsp")
                    for th2 in range(2):
                        ps = pps.tile([56, 448], F32, tag="ps")
                        nc.tensor.matmul(
                            ps[:], M("CWst")[0:64],
                            Z9[:, cb, th2*8:(th2+1)*8, :].rearrange(
                                "p t h -> p (t h)"),
                            start=True, stop=True)
                        nc.scalar.activation(
                            gsp[:, :, th2*8:(th2+1)*8].rearrange("p h t -> p t h"),
                            ps[:].rearrange("p (t h) -> p t h", t=8), AF.Copy)
                    nc.sync.dma_start(
                        Gsp_d[conv].rearrange("w (cb n) -> w cb n", cb=S*B)[:, cb, :],
                        gsp[:].rearrange("p h t -> p (h t)"))
            pgs.release()
            pZ9.release()
            pin.release()

            # readback in scan layout [128=(c,b,hg) | (w, hl, t)]
            dst_sc = g_sc if conv == 0 else h_sc
            nc.sync.dma_start(
                dst_sc[:].rearrange("p (w n) -> p w n", w=W),
                Gsp_d[conv].rearrange("w (p n) -> p w n", p=128))

        # ---------------- Scan ----------------
        with tc.tile_pool(name="scan", bufs=1) as psn:
            Ft = psn.tile([128, NTOK], BF16)
            Vt = psn.tile([128, NTOK], BF16)
            Cs = psn.tile([128, NTOK], BF16)
            nc.scalar.activation(Ft[:], g_sc[:], AF.Sigmoid, scale=-1.0)
            nc.vector.memset(
                Ft[:].rearrange("p (n t) -> p n t", t=T)[:, :, 0:1], 0.0)
            nc.scalar.activation(Vt[:], g_sc[:], AF.Sigmoid)
            nc.vector.tensor_tensor(Cs[:], h_sc[:], h_sc[:], OP.mult)
            nc.vector.tensor_tensor(Vt[:], Vt[:], Cs[:], OP.mult)
            nc.vector.tensor_tensor_scan(Cs[:], Ft[:], Vt[:], 0.0, OP.mult, OP.add)
            nc.sync.dma_start(
                a2a_in_d[:].rearrange("j s n -> s j n"), Cs[:])
        psc.release()

        nc.gpsimd.collective_compute(
            "AllToAll", OP.bypass,
            ins=[a2a_in_d[:].opt()],
            outs=[a2a_out_d[:].opt()],
            replica_groups=[list(range(NCORES))],
        )

        # ---------------- Phase 3 ----------------
        with (
            tc.tile_pool(name="p3", bufs=1) as p3,
            tc.tile_pool(name="p3c", bufs=3) as p3c,
        ):
            Cr = p3.tile([C, NTOK], BF16)
            nc.sync.dma_start(Cr[:], a2a_out_d[:])
            wo = p3.tile([C, C], BF16)
            nc.sync.dma_start(wo[:], wo_d[:])
            w1 = p3.tile([C, HID], BF16)
            nc.sync.dma_start(w1[:], w1_d[:])
            w2 = p3.tile([C, HID], BF16)
            nc.sync.dma_start(w2[:], w2l_d[:])
            onek = p3.tile([128, 1], BF16)
            nc.vector.memset(onek[:], 1.0 / 128.0)
            one1 = p3.tile([1, 128], BF16)
            nc.vector.memset(one1[:], 1.0)
            epsb = p3.tile([1, 1], F32)
            nc.vector.memset(epsb[:], 1e-6)

            ssm = p3.tile([C, NTOK], BF16)
            MB = p3.tile([C, NTOK], BF16)
            RB = p3.tile([C, NTOK], BF16)
            NCH = NTOK // 448
            with (
                tc.tile_pool(name="p3ps1", bufs=2, space="PSUM") as pA,
                tc.tile_pool(name="p3ps2", bufs=1, space="PSUM") as pB,
            ):
                for ch in range(NCH):
                    sl = slice(ch*448, (ch+1)*448)
                    ps = pA.tile([C, 448], F32, tag="wo")
                    nc.tensor.matmul(ps[:], wo[:], Cr[:, sl], start=True, stop=True)
                    nc.scalar.activation(ssm[:, sl], ps[:], AF.Identity, bias=bia[:, 1:2])
                    sq = p3c.tile([C, 448], BF16, tag="sq")
                    nc.scalar.activation(sq[:], ssm[:, sl], AF.Square)
                    psm = pB.tile([1, 448], F32, tag="m")
                    psq = pB.tile([1, 448], F32, tag="q")
                    nc.tensor.matmul(psm[:], onek[:], ssm[:, sl], start=True, stop=True)
                    nc.tensor.matmul(psq[:], onek[:], sq[:], start=True, stop=True)
                    mr = p3c.tile([1, 448], BF16, tag="mr")
                    nc.vector.tensor_copy(mr[:], psm[:])
                    v1 = p3c.tile([1, 448], F32, tag="v1")
                    nc.vector.tensor_tensor(v1[:], psm[:], mr[:], OP.mult)
                    nc.vector.tensor_tensor(v1[:], psq[:], v1[:], OP.subtract)
                    sd = p3c.tile([1, 448], F32, tag="sd")
                    nc.scalar.activation(sd[:], v1[:], AF.Sqrt, bias=epsb[:])
                    rsf = p3c.tile([1, 448], F32, tag="rsf")
                    nc.vector.reciprocal(rsf[:], sd[:])
                    rs = p3c.tile([1, 448], BF16, tag="rs")
                    nc.vector.tensor_copy(rs[:], rsf[:])
                    psbm = pB.tile([128, 448], F32, tag="bm")
                    psbr = pB.tile([128, 448], F32, tag="br")
                    nc.tensor.matmul(psbm[:], one1[:], mr[:], start=True, stop=True)
                    nc.tensor.matmul(psbr[:], one1[:], rs[:], start=True, stop=True)
                    nc.vector.tensor_copy(MB[:, sl], psbm[:])
                    nc.vector.tensor_copy(RB[:, sl], psbr[:])

            yt = p3.tile([C, NTOK], BF16)
            with tc.tile_pool(name="p3ps3", bufs=2, space="PSUM") as pC:
                for ch in range(NCH):
                    sl = slice(ch*448, (ch+1)*448)
                    d = p3c.tile([C, 448], BF16, tag="xn1")
                    nc.vector.tensor_tensor(d[:], ssm[:, sl], MB[:, sl], OP.subtract)
                    nc.vector.tensor_tensor(d[:], d[:], RB[:, sl], OP.mult)
                    xn = p3c.tile([C, 448], BF16, tag="xn2")
                    nc.vector.tensor_scalar(
                        out=xn[:], in0=d[:], scalar1=bia[:, 7:8],
                        scalar2=bia[:, 8:9], op0=OP.mult, op1=OP.add)
                    hts = p3c.tile([C, 4, 448], BF16, tag="ht")
                    hp = p3c.tile([C, 4, 448], BF16, tag="hp")
                    for j in range(4):
                        psh = pC.tile([C, 448], F32, tag="m1")
                        nc.tensor.matmul(psh[:], w1[:, j*128:(j+1)*128], xn[:],
                                         start=True, stop=True)
                        nc.scalar.activation(hp[:, j, :], psh[:],
                                             AF.Identity, bias=bia[:, 2+j:3+j])
                        nc.scalar.activation(hts[:, j, :], psh[:], AF.Sigmoid,
                                             bias=bia[:, 9+j:10+j], scale=1.702)
                    nc.vector.tensor_tensor(
                        hts[:].rearrange("p j n -> p (j n)"),
                        hts[:].rearrange("p j n -> p (j n)"),
                        hp[:].rearrange("p j n -> p (j n)"), OP.mult)
                    psy = pC.tile([C, 448], F32, tag="m2")
                    for j in range(4):
                        nc.tensor.matmul(psy[:], w2[:, j*128:(j+1)*128], hts[:, j, :],
                                         start=(j == 0), stop=(j == 3))
                    nc.scalar.activation(yt[:, sl], psy[:], AF.Identity, bias=bia[:, 6:7])

            NTC = NTOK // 128
            with (
                tc.tile_pool(name="p3ps4", bufs=2, space="PSUM") as pD,
                tc.tile_pool(name="p3x", bufs=2) as p3x,
            ):
                for k0 in range(0, NTC, 7):
                    nn_ = min(7, NTC - k0)
                    xrs = p3x.tile([128, 7, 128], F32, tag="xrs")
                    nc.sync.dma_start(
                        xrs[:, 0:nn_, :],
                        xres_d[:].rearrange("(n p) c -> p n c", p=128)[:, k0:k0+nn_, :])
                    yo = p3x.tile([128, 7, 128], F32, tag="yo")
                    for q in range(nn_):
                        ch = k0 + q
                        pst = pD.tile([128, 128], BF16, tag="t")
                        nc.tensor.transpose(pst[:], yt[:, ch*128:(ch+1)*128], ident[:])
                        nc.vector.tensor_tensor(yo[:, q, :], pst[:], xrs[:, q, :],
                                                OP.add)
                    nc.sync.dma_start(
                        y_d[:].rearrange("(n p) c -> p n c", p=128)[:, k0:k0+nn_, :],
                        yo[:, 0:nn_, :])
    _split_multi_waits(nc)
    return nc


def _split_multi_waits(nc):
    """TRN2 TPB instructions carry exactly one sync wait; hoist extras
    onto preceding same-engine NoOps."""
    n = [0]
    for f in nc.m.functions:
        for blk in f.blocks:
            insts = blk.instructions
            i = 0
            while i < len(insts):
                inst = insts[i]
                si = inst.sync_info
                if si is not None and len(si.on_wait) > 1:
                    waits = list(si.on_wait)
                    for w in waits[:-1]:
                        nop = mybir.InstNoOp(
                            name=f"I-wsplit-{n[0]}", ins=[], outs=[])
                        n[0] += 1
                        nop.engine = inst.engine
                        nop.sync_info = mybir.SyncInfo(
                            on_wait=[w], on_update=[])
                        nc.register_instruction(nop)
                        insts.insert(i, nop)
                        i += 1
                    si.on_wait = [waits[-1]]
                i += 1


# ---------------- entry point ----------------
_NC_CACHE = [None]
LAST_RESULTS = None


def kernel(**inputs):
    global LAST_RESULTS
    import concourse.bass_utils as bass_utils
    if _NC_CACHE[0] is None:
        _NC_CACHE[0] = build_nc()
    nc = _NC_CACHE[0]
    in_maps = host_prep(inputs)
    res = bass_utils.run_bass_kernel_spmd(
        nc, in_maps, core_ids=list(range(NCORES)))
    LAST_RESULTS = res
    return assemble_output(res.results)
